# revision 41
# baseline (speedup 1.0000x reference)
"""Trainium2 Bass kernel for nn_AlignSubNet (ragged per-sample average pooling).

Strategy:
  - text_x is a pure passthrough -> returned host-side, no device work.
  - audio/video alignment: per sample b, out[j] = mean over pool slots of
    segment j-1 (ps = ceil(len/min_len)), expressed as W[64,T] @ x[T,256]
    on the TensorEngine.  W is built on-device from tiny host-precomputed
    index columns (r[t] = output row of frame t, or -1 if t >= len) via a
    fused tensor_scalar (is_equal, mult) per 128-frame slab.
  - Ragged truncation: only ceil(len/128) slabs of each sample are shipped,
    loaded, and matmul'ed.  Audio and video samples are independently
    sorted by slab count and dealt across (core, slot) so a single set of
    compile-time per-slot slab budgets is near-tight for all 8 cores; the
    compiled program is identical on every core (pure SPMD) and all
    data-dependence lives in input data.  Host unpermutes the outputs.
  - TensorE column packing: the two samples of a slot pair run as
    concurrent M=64 matmuls in col-groups (0,0)/(0,64), accumulating in
    separate PSUM banks of one [128,1024] tile, halving PE occupancy;
    ScalarE (ACT) drains PSUM while VectorE builds weights.
"""

import sys

import numpy as np

if "/opt/trn_rl_repo" not in sys.path:
    sys.path.insert(0, "/opt/trn_rl_repo")

B = 128
NCORES = 8
SPC = B // NCORES  # samples (slots) per core
NPAIR = SPC // 2  # slot pairs per core
TA, TV = 1024, 512
D = 256
J = 64  # dst_len
KA, KV = TA // 128, TV // 128  # max slabs per sample
GRP = 4  # slots per output DMA group (2 pairs)

_LAST_RESULTS = None  # BassKernelResults of the last run (for test harness)
_PROGRAM_CACHE = {}  # (na, nv) -> compiled Bass program


def _assign(slabs):
    """Sort samples by slab count (desc) and deal across (core, slot).

    Returns (sample[slot][core] index array [SPC, NCORES], budget[slot]).
    Slot s takes sorted ranks [8s, 8s+8); its budget is the max slab count
    in that window (= the first, sorted desc).  This is the minimum
    possible sum of core-shared slot budgets.  Within a pair (2t, 2t+1)
    the even slot has the larger budget; its extra matmul iterations run
    solo in col-group 0.
    """
    order = np.argsort(-slabs, kind="stable")
    sample = np.empty((SPC, NCORES), dtype=np.int64)
    budget = np.empty(SPC, dtype=np.int64)
    for s in range(SPC):
        win = order[NCORES * s : NCORES * s + NCORES]
        budget[s] = slabs[win[0]]
        for c in range(NCORES):
            sample[s, c] = win[c]
    return sample, budget


def _meta_cols(x_len, min_len, n_slabs):
    """[128, n_slabs+1] f32: r columns for frames t = p*n_slabs + i, then 1/ps."""
    ps = -(-x_len // min_len)
    n = 128 * n_slabs
    t = np.arange(n)
    r = np.where(t < x_len, 1 + t // ps, -1).astype(np.float32)
    out = np.empty((128, n_slabs + 1), dtype=np.float32)
    out[:, :n_slabs] = r.reshape(128, n_slabs)
    out[:, n_slabs] = np.float32(1.0 / ps)
    return out


def _build_program(na, nv):
    """na/nv: per-slot slab budgets (len SPC, pair-equal)."""
    import concourse.bass as bass
    import concourse.bacc as bacc
    import concourse.mybir as mybir
    import concourse.tile as tile
    from concourse.bass_interp import get_hw_module

    f32 = mybir.dt.float32
    nc = bacc.Bacc(
        "TRN2", target_bir_lowering=False, debug=False, num_devices=NCORES
    )

    tot_a = int(sum(na))
    tot_v = int(sum(nv))
    # cst = [jmat | audio meta | video meta] along the free axis: one DMA,
    # one tile -> every weight-build op has a single upstream dependency.
    offs_a = [J + int(sum(na[:s])) + s for s in range(SPC)]
    offs_v = [J + tot_a + SPC + int(sum(nv[:s])) + s for s in range(SPC)]
    NCST = J + tot_a + SPC + tot_v + SPC
    row_a = [128 * int(sum(na[:s])) for s in range(SPC)]
    row_v = [128 * int(sum(nv[:s])) for s in range(SPC)]

    axp = nc.dram_tensor("axp", [128 * tot_a, D], f32, kind="ExternalInput")
    vxp = nc.dram_tensor("vxp", [128 * tot_v, D], f32, kind="ExternalInput")
    cst = nc.dram_tensor("cst", [128, NCST], f32, kind="ExternalInput")
    ao = nc.dram_tensor("ao", [SPC, J, D], f32, kind="ExternalOutput")
    vo = nc.dram_tensor("vo", [SPC, J, D], f32, kind="ExternalOutput")

    with tile.TileContext(nc) as tc:
        with (
            tc.tile_pool(name="const", bufs=1) as cpool,
            tc.tile_pool(name="xa", bufs=4) as xap,
            tc.tile_pool(name="xv", bufs=4) as xvp,
            tc.tile_pool(name="wt", bufs=8) as wtp,
            tc.tile_pool(name="out", bufs=2) as outp,
            tc.tile_pool(name="ps", bufs=2, space=bass.MemorySpace.PSUM) as psp,
        ):
            # const load on the (otherwise idle) SWDGE path so the SP ring
            # starts streaming sample data immediately
            ct = cpool.tile([128, NCST], f32)
            nc.gpsimd.dma_start(ct[:], cst[:])
            jt = ct[:, 0:J]

            def do_pair(pair, kind):
                """One slot pair of one modality: 2 loads, W build, paired
                matmuls (the longer even slot finishes solo), return psum."""
                s0 = 2 * pair
                if kind == "a":
                    bud, xp, rows, offs, pool = na, axp, row_a, offs_a, xap
                else:
                    bud, xp, rows, offs, pool = nv, vxp, row_v, offs_v, xvp
                nb0, nb1 = int(bud[s0]), int(bud[s0 + 1])
                # per-slot loads (budgets differ); audio on the SP HWDGE
                # ring, video on the ACT ring.
                dma_eng = nc.sync if kind == "a" else nc.scalar
                w_eng = nc.vector
                xts = []
                wts = []
                for u, nb in ((0, nb0), (1, nb1)):
                    s = s0 + u
                    xt = pool.tile([128, nb, D], f32, tag=f"{kind}{u}")
                    dma_eng.dma_start(
                        xt[:],
                        xp[rows[s] : rows[s] + 128 * nb].rearrange(
                            "(p i) d -> p i d", p=128
                        ),
                    )
                    xts.append(xt)
                    wt = wtp.tile([128, nb, J], f32, tag=f"w{kind}{u}")
                    base = offs[s]
                    for i in range(nb):
                        w_eng.tensor_scalar(
                            wt[:, i, :],
                            jt,
                            ct[:, base + i : base + i + 1],
                            ct[:, base + nb : base + nb + 1],
                            mybir.AluOpType.is_equal,
                            mybir.AluOpType.mult,
                        )
                    wts.append(wt)
                # Col-group halves accumulate in SEPARATE psum banks (free
                # offsets 0 and 512 f32) so each is an independent, clean
                # accumulation group.
                pt = psp.tile([128, 1024], f32, tag="p" + kind)
                for i in range(nb0):
                    nc.tensor.matmul(
                        pt[0:J, 0:D],
                        wts[0][:, i, :],
                        xts[0][:, i, :],
                        start=(i == 0),
                        stop=(i == nb0 - 1),
                        tile_position=(0, 0),
                    )
                    if i < nb1:
                        nc.tensor.matmul(
                            pt[J:128, 512 : 512 + D],
                            wts[1][:, i, :],
                            xts[1][:, i, :],
                            start=(i == 0),
                            stop=(i == nb1 - 1),
                            tile_position=(0, J),
                        )
                return pt

            for g in range(SPC // GRP):
                oa = outp.tile([128, GRP // 2, D], f32, tag="oa")
                ov = outp.tile([128, GRP // 2, D], f32, tag="ov")
                for p2 in range(GRP // 2):
                    pair = g * (GRP // 2) + p2
                    pa = do_pair(pair, "a")
                    nc.scalar.copy(oa[0:J, p2, :], pa[0:J, 0:D])
                    nc.scalar.copy(oa[J:128, p2, :], pa[J:128, 512 : 512 + D])
                    pv = do_pair(pair, "v")
                    nc.scalar.copy(ov[0:J, p2, :], pv[0:J, 0:D])
                    nc.scalar.copy(ov[J:128, p2, :], pv[J:128, 512 : 512 + D])

                # packed out-DMA: partition = (slot%2)*64 + j, free = (pair, d)
                nc.sync.dma_start(
                    ao[g * GRP : (g + 1) * GRP].rearrange(
                        "(s2 s1) j d -> (s1 j) s2 d", s1=2
                    ),
                    oa[:],
                )
                nc.sync.dma_start(
                    vo[g * GRP : (g + 1) * GRP].rearrange(
                        "(s2 s1) j d -> (s1 j) s2 d", s1=2
                    ),
                    ov[:],
                )

    nc.compile()  # bacc passes: wait legalization (1-wait/inst on TRN2), DCE, regalloc
    nc.m = get_hw_module(nc.m)
    return nc


def _prep(x_full, lengths, min_len, sample, budgets):
    """Per-core packed input arrays + meta column blocks."""
    packed = []
    metas = []
    for c in range(NCORES):
        chunks = []
        mcols = []
        for s in range(SPC):
            b = int(budgets[s])
            idx = int(sample[s, c])
            chunks.append(x_full[idx, : 128 * b])
            mcols.append(_meta_cols(int(lengths[idx]), int(min_len[idx]), b))
        packed.append(np.ascontiguousarray(np.concatenate(chunks, axis=0)))
        metas.append(np.concatenate(mcols, axis=1))
    return packed, metas


def kernel(
    text_x,
    audio_x,
    video_x,
    text_lengths,
    audio_lengths,
    video_lengths,
    _trace=False,
):
    global _LAST_RESULTS
    from concourse.bass_utils import run_bass_kernel_spmd

    text_x = np.ascontiguousarray(np.asarray(text_x, dtype=np.float32))
    audio_x = np.ascontiguousarray(np.asarray(audio_x, dtype=np.float32))
    video_x = np.ascontiguousarray(np.asarray(video_x, dtype=np.float32))
    text_lengths = np.asarray(text_lengths).astype(np.int64)
    audio_lengths = np.asarray(audio_lengths).astype(np.int64)
    video_lengths = np.asarray(video_lengths).astype(np.int64)

    min_len = text_lengths - 2
    a_slabs = -(-audio_lengths // 128)
    v_slabs = -(-video_lengths // 128)
    sample_a, bud_a = _assign(a_slabs)
    sample_v, bud_v = _assign(v_slabs)
    na = [int(bud_a[s]) for s in range(SPC)]
    nv = [int(bud_v[s]) for s in range(SPC)]

    key = (tuple(na), tuple(nv))
    nc = _PROGRAM_CACHE.get(key)
    if nc is None:
        nc = _PROGRAM_CACHE[key] = _build_program(na, nv)

    packed_a, meta_a = _prep(audio_x, audio_lengths, min_len, sample_a, bud_a)
    packed_v, meta_v = _prep(video_x, video_lengths, min_len, sample_v, bud_v)
    jmat = np.ascontiguousarray(
        np.broadcast_to(np.arange(J, dtype=np.float32), (128, J))
    )

    in_maps = []
    for c in range(NCORES):
        in_maps.append(
            {
                "axp": packed_a[c],
                "vxp": packed_v[c],
                "cst": np.ascontiguousarray(
                    np.concatenate([jmat, meta_a[c], meta_v[c]], axis=1)
                ),
            }
        )

    res = run_bass_kernel_spmd(nc, in_maps, list(range(NCORES)), trace=_trace)
    _LAST_RESULTS = res

    audio_out = np.empty((B, J, D), dtype=np.float32)
    video_out = np.empty((B, J, D), dtype=np.float32)
    for c in range(NCORES):
        ao = res.results[c]["ao"]
        vo = res.results[c]["vo"]
        for s in range(SPC):
            audio_out[sample_a[s, c]] = ao[s]
            video_out[sample_v[s, c]] = vo[s]
    return text_x, audio_out, video_out


# revision 44
# speedup vs baseline: 1.0330x; 1.0330x over previous
"""Trainium2 Bass kernel for nn_AlignSubNet (ragged per-sample average pooling).

Strategy:
  - text_x is a pure passthrough -> returned host-side, no device work.
  - audio/video alignment: per sample b, out[j] = mean over pool slots of
    segment j-1 (ps = ceil(len/min_len)), expressed as W[64,T] @ x[T,256]
    on the TensorEngine.  W is built on-device from tiny host-precomputed
    index columns (r[t] = output row of frame t, or -1 if t >= len) via a
    fused tensor_scalar (is_equal, mult) per 128-frame slab.
  - Ragged truncation: only ceil(len/128) slabs of each sample are shipped,
    loaded, and matmul'ed.  Audio and video samples are independently
    sorted by slab count and dealt across (core, slot) so a single set of
    compile-time per-slot slab budgets is near-tight for all 8 cores; the
    compiled program is identical on every core (pure SPMD) and all
    data-dependence lives in input data.  Host unpermutes the outputs.
  - TensorE column packing: the two samples of a slot pair run as
    concurrent M=64 matmuls in col-groups (0,0)/(0,64), accumulating in
    separate PSUM banks of one [128,1024] tile, halving PE occupancy;
    ScalarE (ACT) drains PSUM while VectorE builds weights.
"""

import sys

import numpy as np

if "/opt/trn_rl_repo" not in sys.path:
    sys.path.insert(0, "/opt/trn_rl_repo")

B = 128
NCORES = 8
SPC = B // NCORES  # samples (slots) per core
NPAIR = SPC // 2  # slot pairs per core
TA, TV = 1024, 512
D = 256
J = 64  # dst_len
KA, KV = TA // 128, TV // 128  # max slabs per sample
GRP = 4  # slots per output DMA group (2 pairs)

_LAST_RESULTS = None  # BassKernelResults of the last run (for test harness)
_PROGRAM_CACHE = {}  # (na, nv) -> compiled Bass program


def _assign(slabs):
    """Sort samples by slab count (desc) and deal across (core, slot).

    Returns (sample[slot][core] index array [SPC, NCORES], budget[slot]).
    Slot s takes sorted ranks [8s, 8s+8); its budget is the max slab count
    in that window (= the first, sorted desc).  This is the minimum
    possible sum of core-shared slot budgets.  Within a pair (2t, 2t+1)
    the even slot has the larger budget; its extra matmul iterations run
    solo in col-group 0.
    """
    order = np.argsort(-slabs, kind="stable")
    sample = np.empty((SPC, NCORES), dtype=np.int64)
    budget = np.empty(SPC, dtype=np.int64)
    for s in range(SPC):
        win = order[NCORES * s : NCORES * s + NCORES]
        budget[s] = slabs[win[0]]
        for c in range(NCORES):
            sample[s, c] = win[c]
    return sample, budget


def _meta_cols(x_len, min_len, n_slabs):
    """[128, n_slabs+1] f32: r columns for frames t = p*n_slabs + i, then 1/ps."""
    ps = -(-x_len // min_len)
    n = 128 * n_slabs
    t = np.arange(n)
    r = np.where(t < x_len, 1 + t // ps, -1).astype(np.float32)
    out = np.empty((128, n_slabs + 1), dtype=np.float32)
    out[:, :n_slabs] = r.reshape(128, n_slabs)
    out[:, n_slabs] = np.float32(1.0 / ps)
    return out


def _build_program(na, nv):
    """na/nv: per-slot slab budgets (len SPC, pair-equal)."""
    import concourse.bass as bass
    import concourse.bacc as bacc
    import concourse.mybir as mybir
    import concourse.tile as tile
    from concourse.bass_interp import get_hw_module

    f32 = mybir.dt.float32
    nc = bacc.Bacc(
        "TRN2", target_bir_lowering=False, debug=False, num_devices=NCORES
    )

    tot_a = int(sum(na))
    tot_v = int(sum(nv))
    # cst = [jmat | audio meta | video meta] along the free axis: one DMA,
    # one tile -> every weight-build op has a single upstream dependency.
    offs_a = [J + int(sum(na[:s])) + s for s in range(SPC)]
    offs_v = [J + tot_a + SPC + int(sum(nv[:s])) + s for s in range(SPC)]
    NCST = J + tot_a + SPC + tot_v + SPC
    row_a = [128 * int(sum(na[:s])) for s in range(SPC)]
    row_v = [128 * int(sum(nv[:s])) for s in range(SPC)]

    axp = nc.dram_tensor("axp", [128 * tot_a, D], f32, kind="ExternalInput")
    vxp = nc.dram_tensor("vxp", [128 * tot_v, D], f32, kind="ExternalInput")
    cst = nc.dram_tensor("cst", [128, NCST], f32, kind="ExternalInput")
    # outputs in the SBUF tile's own layout [group, (s1 j), s2, d] -> 2KB
    # contiguous per partition per out-DMA; the host un-permutes (it already
    # un-permutes samples anyway)
    NG = SPC // GRP
    ao = nc.dram_tensor("ao", [NG, 128, GRP // 2, D], f32, kind="ExternalOutput")
    vo = nc.dram_tensor("vo", [NG, 128, GRP // 2, D], f32, kind="ExternalOutput")

    with tile.TileContext(nc) as tc:
        with (
            tc.tile_pool(name="const", bufs=1) as cpool,
            tc.tile_pool(name="xa", bufs=4) as xap,
            tc.tile_pool(name="xv", bufs=4) as xvp,
            tc.tile_pool(name="wt", bufs=8) as wtp,
            tc.tile_pool(name="out", bufs=2) as outp,
            tc.tile_pool(name="ps", bufs=2, space=bass.MemorySpace.PSUM) as psp,
        ):
            # const load on the (otherwise idle) SWDGE path so the SP ring
            # starts streaming sample data immediately
            ct = cpool.tile([128, NCST], f32)
            nc.gpsimd.dma_start(ct[:], cst[:])
            jt = ct[:, 0:J]

            def do_pair(pair, kind):
                """One slot pair of one modality: 2 loads, W build, paired
                matmuls (the longer even slot finishes solo), return psum."""
                s0 = 2 * pair
                if kind == "a":
                    bud, xp, rows, offs, pool = na, axp, row_a, offs_a, xap
                else:
                    bud, xp, rows, offs, pool = nv, vxp, row_v, offs_v, xvp
                nb0, nb1 = int(bud[s0]), int(bud[s0 + 1])
                # per-slot loads (budgets differ); audio on the SP HWDGE
                # ring, video on the ACT ring.
                dma_eng = nc.sync if kind == "a" else nc.scalar
                w_eng = nc.vector
                xts = []
                wts = []
                for u, nb in ((0, nb0), (1, nb1)):
                    s = s0 + u
                    xt = pool.tile([128, nb, D], f32, tag=f"{kind}{u}")
                    dma_eng.dma_start(
                        xt[:],
                        xp[rows[s] : rows[s] + 128 * nb].rearrange(
                            "(p i) d -> p i d", p=128
                        ),
                    )
                    xts.append(xt)
                    wt = wtp.tile([128, nb, J], f32, tag=f"w{kind}{u}")
                    base = offs[s]
                    for i in range(nb):
                        w_eng.tensor_scalar(
                            wt[:, i, :],
                            jt,
                            ct[:, base + i : base + i + 1],
                            ct[:, base + nb : base + nb + 1],
                            mybir.AluOpType.is_equal,
                            mybir.AluOpType.mult,
                        )
                    wts.append(wt)
                # Col-group halves accumulate in SEPARATE psum banks (free
                # offsets 0 and 512 f32) so each is an independent, clean
                # accumulation group.
                pt = psp.tile([128, 1024], f32, tag="p" + kind)
                for i in range(nb0):
                    nc.tensor.matmul(
                        pt[0:J, 0:D],
                        wts[0][:, i, :],
                        xts[0][:, i, :],
                        start=(i == 0),
                        stop=(i == nb0 - 1),
                        tile_position=(0, 0),
                    )
                    if i < nb1:
                        nc.tensor.matmul(
                            pt[J:128, 512 : 512 + D],
                            wts[1][:, i, :],
                            xts[1][:, i, :],
                            start=(i == 0),
                            stop=(i == nb1 - 1),
                            tile_position=(0, J),
                        )
                return pt

            for g in range(NG):
                # video pairs processed big-last (group NG-1-g) so the late
                # DMA stream stays fat as the descending audio stream thins
                gv = NG - 1 - g
                oa = outp.tile([128, GRP // 2, D], f32, tag="oa")
                ov = outp.tile([128, GRP // 2, D], f32, tag="ov")
                for p2 in range(GRP // 2):
                    pa = do_pair(g * (GRP // 2) + p2, "a")
                    nc.scalar.copy(oa[0:J, p2, :], pa[0:J, 0:D])
                    nc.scalar.copy(oa[J:128, p2, :], pa[J:128, 512 : 512 + D])
                    pv = do_pair(gv * (GRP // 2) + p2, "v")
                    nc.scalar.copy(ov[0:J, p2, :], pv[0:J, 0:D])
                    nc.scalar.copy(ov[J:128, p2, :], pv[J:128, 512 : 512 + D])

                nc.sync.dma_start(ao[g], oa[:])
                nc.sync.dma_start(vo[gv], ov[:])

    nc.compile()  # bacc passes: wait legalization (1-wait/inst on TRN2), DCE, regalloc
    nc.m = get_hw_module(nc.m)
    return nc


def _prep(x_full, lengths, min_len, sample, budgets):
    """Per-core packed input arrays + meta column blocks."""
    packed = []
    metas = []
    for c in range(NCORES):
        chunks = []
        mcols = []
        for s in range(SPC):
            b = int(budgets[s])
            idx = int(sample[s, c])
            chunks.append(x_full[idx, : 128 * b])
            mcols.append(_meta_cols(int(lengths[idx]), int(min_len[idx]), b))
        packed.append(np.ascontiguousarray(np.concatenate(chunks, axis=0)))
        metas.append(np.concatenate(mcols, axis=1))
    return packed, metas


def kernel(
    text_x,
    audio_x,
    video_x,
    text_lengths,
    audio_lengths,
    video_lengths,
    _trace=False,
):
    global _LAST_RESULTS
    from concourse.bass_utils import run_bass_kernel_spmd

    text_x = np.ascontiguousarray(np.asarray(text_x, dtype=np.float32))
    audio_x = np.ascontiguousarray(np.asarray(audio_x, dtype=np.float32))
    video_x = np.ascontiguousarray(np.asarray(video_x, dtype=np.float32))
    text_lengths = np.asarray(text_lengths).astype(np.int64)
    audio_lengths = np.asarray(audio_lengths).astype(np.int64)
    video_lengths = np.asarray(video_lengths).astype(np.int64)

    min_len = text_lengths - 2
    a_slabs = -(-audio_lengths // 128)
    v_slabs = -(-video_lengths // 128)
    sample_a, bud_a = _assign(a_slabs)
    sample_v, bud_v = _assign(v_slabs)
    na = [int(bud_a[s]) for s in range(SPC)]
    nv = [int(bud_v[s]) for s in range(SPC)]

    key = (tuple(na), tuple(nv))
    nc = _PROGRAM_CACHE.get(key)
    if nc is None:
        nc = _PROGRAM_CACHE[key] = _build_program(na, nv)

    packed_a, meta_a = _prep(audio_x, audio_lengths, min_len, sample_a, bud_a)
    packed_v, meta_v = _prep(video_x, video_lengths, min_len, sample_v, bud_v)
    jmat = np.ascontiguousarray(
        np.broadcast_to(np.arange(J, dtype=np.float32), (128, J))
    )

    in_maps = []
    for c in range(NCORES):
        in_maps.append(
            {
                "axp": packed_a[c],
                "vxp": packed_v[c],
                "cst": np.ascontiguousarray(
                    np.concatenate([jmat, meta_a[c], meta_v[c]], axis=1)
                ),
            }
        )

    res = run_bass_kernel_spmd(nc, in_maps, list(range(NCORES)), trace=_trace)
    _LAST_RESULTS = res

    audio_out = np.empty((B, J, D), dtype=np.float32)
    video_out = np.empty((B, J, D), dtype=np.float32)
    for c in range(NCORES):
        # device layout [group, (s1 j), s2, d] -> slot = g*GRP + s2*2 + s1
        ao = _unpack_out(res.results[c]["ao"])
        vo = _unpack_out(res.results[c]["vo"])
        for s in range(SPC):
            audio_out[sample_a[s, c]] = ao[s]
            video_out[sample_v[s, c]] = vo[s]
    return text_x, audio_out, video_out


def _unpack_out(arr):
    """[NG, 128, GRP//2, D] device layout -> [SPC, J, D] slot order."""
    ng = SPC // GRP
    return (
        arr.reshape(ng, 2, J, GRP // 2, D)
        .transpose(0, 3, 1, 2, 4)
        .reshape(SPC, J, D)
    )
